# revision 6
# baseline (speedup 1.0000x reference)
"""Butterfly-Conv2d (nn_BConv2d) Trainium2 kernel.

Math (reference): x(B=64,IC=16,32,32) -> y=x.reshape(IC,B,N=1024)[:,:,bitrev];
broadcast over OC=32; 10 radix-2 butterfly layers with per-(ic,oc) twiddles;
mean over ic; + bias -> (B,OC,32,32).

Strategy:
  * Shard over OC: 8 cores x 4 oc each; every core holds all 16 ic so the
    ic-mean is core-local (no collective). Host concatenates oc slices.
  * Weight transform (host, layout/compose only -- analogous to a Winograd
    filter transform): butterfly layers 0..7 compose into dense 256x256
    block-diagonal matrices (4 blocks per (ic,oc)). TensorE applies them as
    K=128 matmuls with PSUM K-accumulation (16 MMs per (ic,oc), N=64).
  * Layers 8,9 pair elements across 512/256 strides = across free-dim chunks
    in the device layout [128 partitions = n%128, free = (chunk n//128, b)].
    VectorE does them with scalar_tensor_tensor using per-partition twiddle
    vectors ([P,1] APs, no broadcast/replication needed), then accumulates
    the ic-mean (1/16 folded into layer-9 coeffs) and bias.

Device layout: y[ic] tile [128, 512]: partition p = n & 127, free = c*64 + b
with chunk c = n >> 7 (3 bits: n9 n8 n7).
"""

import numpy as np

B, IC, OC, H, W = 64, 16, 32, 32, 32
N = H * W          # 1024
M = 10             # butterfly layers
NCORES = 8
OCL = OC // NCORES  # 4 oc per core
NCH = 8            # free-dim chunks (n9n8n7)
P = 128            # partitions (n6..n0)
SB = 256           # composed stage-A block size (layers 0..7)
NBLK = N // SB     # 4 blocks per (ic,oc)

# stage-A (matmul) dtype: np.float32 or ml_dtypes.bfloat16 (set in W_DTYPE)
W_DTYPE = np.float32


def _bitrev(n):
    bits = int(np.log2(n))
    idx = np.arange(n, dtype=np.int64)
    rev = np.zeros(n, dtype=np.int64)
    for b in range(bits):
        rev = (rev << 1) | ((idx >> b) & 1)
    return rev


def _compose_stageA(tw):
    """Compose butterfly layers 0..7 into A[ic,oc,g,256,256] (g=4 blocks).

    Layer l (stride s=2^l) acts on flat index k*2s + q*s + j; for l<=7 the
    mixing stays inside 256-aligned blocks.
    """
    ic, oc = tw.shape[0], tw.shape[1]
    A = np.zeros((ic, oc, NBLK, SB, SB), dtype=np.float32)
    eye = np.eye(SB, dtype=np.float32)
    A[:] = eye  # broadcast
    for l in range(8):
        s = 1 << l
        nb_loc = SB // (2 * s)  # local pair-block count inside a 256 block
        # twiddle layer l: (ic, oc, nb, s, 2, 2) with nb = N//(2s)
        t = tw[:, :, l].reshape(ic, oc, N // (2 * s), s, 2, 2)
        # local slice for block g: global k = g*nb_loc + k_loc
        t = t.reshape(ic, oc, NBLK, nb_loc, s, 2, 2)
        # A view: (ic, oc, g, k_loc, 2, s, SB) rows evolve
        Av = A.reshape(ic, oc, NBLK, nb_loc, 2, s, SB)
        a0 = Av[:, :, :, :, 0]  # (ic,oc,g,k,s,SB)
        a1 = Av[:, :, :, :, 1]
        t00 = t[..., 0, 0, None]  # (ic,oc,g,k,s,1)
        t01 = t[..., 0, 1, None]
        t10 = t[..., 1, 0, None]
        t11 = t[..., 1, 1, None]
        new0 = t00 * a0 + t01 * a1
        new1 = t10 * a0 + t11 * a1
        Av[:, :, :, :, 0] = new0
        Av[:, :, :, :, 1] = new1
    return A


def _stageB_coeffs(tw):
    """Per-partition coefficient vectors for layers 8 and 9.

    Returns tb[ic, oc, 2(layer), 8(out chunk c'), 2(q), 128(p)] float32,
    with the 1/IC mean folded into layer 9, and the input-chunk index map.
    layer 8 (s=256): flat = k*512 + q*256 + j, j=(n7,p);  out chunk
      c' = (k, p_out, n7) -> reads chunks (k,q,n7), coeff t8[k, (n7,p), p_out, q].
    layer 9 (s=512): flat = q*512 + j, j=(n8,n7,p); out c' = (p_out, n8, n7)
      -> reads chunks (q, n8, n7), coeff t9[0, (n8,n7,p), p_out, q]/16.
    """
    ic, oc = tw.shape[0], tw.shape[1]
    t8 = tw[:, :, 8].reshape(ic, oc, 2, 256, 2, 2)   # [k, j, p_out, q]
    t9 = tw[:, :, 9].reshape(ic, oc, 1, 512, 2, 2)
    tb = np.zeros((ic, oc, 2, NCH, 2, P), dtype=np.float32)
    src = np.zeros((2, NCH, 2), dtype=np.int64)
    pr = np.arange(P)
    for cp in range(NCH):
        k, p_out, n7 = cp >> 2, (cp >> 1) & 1, cp & 1
        for q in range(2):
            tb[:, :, 0, cp, q] = t8[:, :, k, n7 * 128 + pr, p_out, q]
            src[0, cp, q] = k * 4 + q * 2 + n7
        p_out9, n8, n7_ = cp >> 2, (cp >> 1) & 1, cp & 1
        for q in range(2):
            tb[:, :, 1, cp, q] = t9[:, :, 0, (cp & 3) * 128 + pr, p_out9, q] / IC
            src[1, cp, q] = q * 4 + (cp & 3)
    return tb, src


_SRC8 = None  # filled lazily (static chunk-index map, twiddle-independent)


def _prep_host(x, twiddle, bias):
    """All host-side layout work. Returns per-core input maps (numpy)."""
    perm = _bitrev(N)
    y = np.ascontiguousarray(x).reshape(IC, B, N)[:, :, perm]
    # device layout y[ic, p, c*64+b]
    y_dev = np.ascontiguousarray(
        y.reshape(IC, B, NCH, P).transpose(0, 3, 2, 1)
    ).reshape(IC, P, NCH * B)

    A = _compose_stageA(np.asarray(twiddle, dtype=np.float32))
    tb, src = _stageB_coeffs(np.asarray(twiddle, dtype=np.float32))

    bias_np = np.asarray(bias, dtype=np.float32).reshape(OC, NCH, P)

    in_maps = []
    for core in range(NCORES):
        osl = slice(core * OCL, (core + 1) * OCL)
        Ac = A[:, osl]  # (IC, OCL, 4, 256, 256)
        # lhsT tiles: w[ic,o,p_k, g, h, kin, m] = Ac[ic,o,g][h*128+m, kin*128+p_k]
        w = np.ascontiguousarray(
            Ac.reshape(IC, OCL, NBLK, 2, P, 2, P)  # [g, h, m, kin, k]
            .transpose(0, 1, 6, 2, 3, 5, 4)        # [ic,o,k,g,h,kin,m]
        ).astype(W_DTYPE)
        tbc = np.ascontiguousarray(
            tb[:, osl].transpose(0, 1, 5, 2, 3, 4)  # [ic,o,p,l,c',q]
        ).reshape(IC, OCL, P, 32).astype(np.float32)
        bc = np.ascontiguousarray(
            np.broadcast_to(
                bias_np[osl].transpose(0, 2, 1)[:, :, :, None], (OCL, P, NCH, B)
            )
        ).reshape(OCL, P, NCH * B).astype(np.float32)
        in_maps.append(
            {
                "y": y_dev.astype(W_DTYPE),
                "w": w.reshape(IC, OCL, P, NBLK * 2 * 2 * P),
                "tb": tbc,
                "bias": bc,
            }
        )
    return in_maps, src


def _emulate_core(im, src):
    """Numpy emulation of the device program (for validating layout math)."""
    y = im["y"].astype(np.float32)      # (IC, 128, 512)
    w = im["w"].astype(np.float32).reshape(IC, OCL, P, NBLK, 2, 2, P)
    tb = im["tb"].reshape(IC, OCL, P, 2, NCH, 2)
    out = np.array(im["bias"], dtype=np.float32).reshape(OCL, P, NCH, B).copy()
    for o in range(OCL):
        for ic in range(IC):
            z = np.zeros((P, NCH, B), dtype=np.float32)
            yv = y[ic].reshape(P, NCH, B)
            for g in range(NBLK):
                for h in range(2):
                    acc = np.zeros((P, B), dtype=np.float32)
                    for kin in range(2):
                        lhsT = w[ic, o, :, g, h, kin]  # [k, m]
                        acc += lhsT.T @ yv[:, 2 * g + kin]
                    z[:, 2 * g + h] = acc
            y8 = np.zeros_like(z)
            for cp in range(NCH):
                y8[:, cp] = (
                    tb[ic, o, :, 0, cp, 0, None] * z[:, src[0, cp, 0]]
                    + tb[ic, o, :, 0, cp, 1, None] * z[:, src[0, cp, 1]]
                )
            for cp in range(NCH):
                out[o, :, cp] += (
                    tb[ic, o, :, 1, cp, 0, None] * y8[:, src[1, cp, 0]]
                    + tb[ic, o, :, 1, cp, 1, None] * y8[:, src[1, cp, 1]]
                )
    return out.reshape(OCL, P, NCH * B)


def _build_program(src):
    import concourse.bacc as bacc
    import concourse.mybir as mybir
    from concourse.tile import TileContext

    wdt = mybir.dt.bfloat16 if W_DTYPE != np.float32 else mybir.dt.float32
    f32 = mybir.dt.float32
    MULT, ADD = mybir.AluOpType.mult, mybir.AluOpType.add

    nc = bacc.Bacc(None, target_bir_lowering=False)
    y_d = nc.dram_tensor("y", (IC, P, NCH * B), wdt, kind="ExternalInput")
    w_d = nc.dram_tensor("w", (IC, OCL, P, NBLK * 4 * P), wdt, kind="ExternalInput")
    tb_d = nc.dram_tensor("tb", (IC, OCL, P, 32), f32, kind="ExternalInput")
    bias_d = nc.dram_tensor("bias", (OCL, P, NCH * B), f32, kind="ExternalInput")
    o_d = nc.dram_tensor("o", (OCL, P, NCH * B), f32, kind="ExternalOutput")

    with TileContext(nc) as tc:
        with (
            tc.tile_pool(name="ypool", bufs=2) as ypool,
            tc.tile_pool(name="wpool", bufs=3) as wpool,
            tc.tile_pool(name="tbpool", bufs=3) as tbpool,
            tc.tile_pool(name="accpool", bufs=OCL) as accpool,
            tc.tile_pool(name="y8pool", bufs=3) as y8pool,
            tc.tile_pool(name="psum", bufs=4, space="PSUM") as pspool,
        ):
            accs = []
            for o in range(OCL):
                acc = accpool.tile([P, NCH * B], f32, tag="acc")
                nc.sync.dma_start(out=acc[:], in_=bias_d[o])
                accs.append(acc)
            for ic in range(IC):
                ytile = ypool.tile([P, NCH * B], wdt)
                nc.sync.dma_start(out=ytile[:], in_=y_d[ic])
                for o in range(OCL):
                    wtile = wpool.tile([P, NBLK * 4 * P], wdt)
                    nc.sync.dma_start(out=wtile[:], in_=w_d[ic, o])
                    tbt = tbpool.tile([P, 32], f32)
                    nc.sync.dma_start(out=tbt[:], in_=tb_d[ic, o])
                    z = pspool.tile([P, NCH * B], f32)
                    for g in range(NBLK):
                        for h in range(2):
                            cp = 2 * g + h
                            for kin in range(2):
                                wi = ((g * 2 + h) * 2 + kin) * P
                                nc.tensor.matmul(
                                    z[:, cp * B : (cp + 1) * B],
                                    wtile[:, wi : wi + P],
                                    ytile[:, (2 * g + kin) * B : (2 * g + kin + 1) * B],
                                    start=(kin == 0),
                                    stop=(kin == 1),
                                )
                    y8 = y8pool.tile([P, NCH * B], f32)
                    for cp in range(NCH):
                        s0, s1 = int(src[0, cp, 0]), int(src[0, cp, 1])
                        osl = slice(cp * B, (cp + 1) * B)
                        # y8_cp = t_q1 * z_s1 ; then y8_cp = (z_s0*t_q0) + y8_cp
                        nc.vector.tensor_scalar_mul(
                            y8[:, osl],
                            z[:, s1 * B : (s1 + 1) * B],
                            tbt[:, (0 * NCH + cp) * 2 + 1 : (0 * NCH + cp) * 2 + 2],
                        )
                        nc.vector.scalar_tensor_tensor(
                            y8[:, osl],
                            z[:, s0 * B : (s0 + 1) * B],
                            tbt[:, (0 * NCH + cp) * 2 : (0 * NCH + cp) * 2 + 1],
                            y8[:, osl],
                            MULT,
                            ADD,
                        )
                    yo = y8pool.tile([P, NCH * B], f32, tag="yo")
                    for cp in range(NCH):
                        s0, s1 = int(src[1, cp, 0]), int(src[1, cp, 1])
                        osl = slice(cp * B, (cp + 1) * B)
                        nc.vector.tensor_scalar_mul(
                            yo[:, osl],
                            y8[:, s1 * B : (s1 + 1) * B],
                            tbt[:, (1 * NCH + cp) * 2 + 1 : (1 * NCH + cp) * 2 + 2],
                        )
                        nc.vector.scalar_tensor_tensor(
                            yo[:, osl],
                            y8[:, s0 * B : (s0 + 1) * B],
                            tbt[:, (1 * NCH + cp) * 2 : (1 * NCH + cp) * 2 + 1],
                            yo[:, osl],
                            MULT,
                            ADD,
                        )
                    nc.vector.tensor_add(accs[o][:], accs[o][:], yo[:])
            for o in range(OCL):
                nc.sync.dma_start(out=o_d[o], in_=accs[o][:])
    nc.finalize()
    return nc


_LAST_RESULTS = {"exec_time_ns": None}


def kernel(x, twiddle, bias, _trace=False, _emulate=False):
    in_maps, src = _prep_host(
        np.asarray(x), np.asarray(twiddle), np.asarray(bias)
    )
    if _emulate:
        outs = [_emulate_core(im, src) for im in in_maps]
    else:
        from concourse.bass_utils import run_bass_kernel_spmd

        nc = _build_program(src)
        res = run_bass_kernel_spmd(
            nc, in_maps, list(range(NCORES)), trace=_trace
        )
        _LAST_RESULTS["exec_time_ns"] = res.exec_time_ns
        _LAST_RESULTS["mean_exec_time_ns"] = res.mean_exec_time_ns
        outs = [r["o"] for r in res.results]
    # o[oc_l, p, c*64+b] -> (OC, B, N); final (B,OC,H,W) is a pure
    # reinterpret of (OC,B,N) bytes (reference uses .reshape, not transpose).
    full = np.concatenate(
        [
            np.asarray(o, dtype=np.float32)
            .reshape(OCL, P, NCH, B)
            .transpose(0, 3, 2, 1)
            .reshape(OCL, B, N)
            for o in outs
        ],
        axis=0,
    )
    return np.ascontiguousarray(full).reshape(B, OC, H, W).astype(np.float32)


# revision 7
# speedup vs baseline: 1.0848x; 1.0848x over previous
"""Butterfly-Conv2d (nn_BConv2d) Trainium2 kernel.

Math (reference): x(B=64,IC=16,32,32) -> y=x.reshape(IC,B,N=1024)[:,:,bitrev];
broadcast over OC=32; 10 radix-2 butterfly layers with per-(ic,oc) twiddles;
mean over ic; + bias -> (B,OC,32,32).

Strategy:
  * Shard over OC: 8 cores x 4 oc each; every core holds all 16 ic so the
    ic-mean is core-local (no collective). Host concatenates oc slices.
  * Weight transform (host, layout/compose only -- analogous to a Winograd
    filter transform): butterfly layers 0..7 compose into dense 256x256
    block-diagonal matrices (4 blocks per (ic,oc)). TensorE applies them as
    K=128 matmuls with PSUM K-accumulation (16 MMs per (ic,oc), N=64).
  * Layers 8,9 pair elements across 512/256 strides = across free-dim chunks
    in the device layout [128 partitions = n%128, free = (chunk n//128, b)].
    VectorE does them with scalar_tensor_tensor using per-partition twiddle
    vectors ([P,1] APs, no broadcast/replication needed), then accumulates
    the ic-mean (1/16 folded into layer-9 coeffs) and bias.

Device layout: y[ic] tile [128, 512]: partition p = n & 127, free = c*64 + b
with chunk c = n >> 7 (3 bits: n9 n8 n7).
"""

import numpy as np

B, IC, OC, H, W = 64, 16, 32, 32, 32
N = H * W          # 1024
M = 10             # butterfly layers
NCORES = 8
OCL = OC // NCORES  # 4 oc per core
NCH = 8            # free-dim chunks (n9n8n7)
P = 128            # partitions (n6..n0)
SB = 256           # composed stage-A block size (layers 0..7)
NBLK = N // SB     # 4 blocks per (ic,oc)

# stage-A (matmul) dtype: np.float32 or ml_dtypes.bfloat16 (set in W_DTYPE)
import ml_dtypes

W_DTYPE = ml_dtypes.bfloat16


def _bitrev(n):
    bits = int(np.log2(n))
    idx = np.arange(n, dtype=np.int64)
    rev = np.zeros(n, dtype=np.int64)
    for b in range(bits):
        rev = (rev << 1) | ((idx >> b) & 1)
    return rev


def _compose_stageA(tw):
    """Compose butterfly layers 0..7 into A[ic,oc,g,256,256] (g=4 blocks).

    Layer l (stride s=2^l) acts on flat index k*2s + q*s + j; for l<=7 the
    mixing stays inside 256-aligned blocks.
    """
    ic, oc = tw.shape[0], tw.shape[1]
    A = np.zeros((ic, oc, NBLK, SB, SB), dtype=np.float32)
    eye = np.eye(SB, dtype=np.float32)
    A[:] = eye  # broadcast
    for l in range(8):
        s = 1 << l
        nb_loc = SB // (2 * s)  # local pair-block count inside a 256 block
        # twiddle layer l: (ic, oc, nb, s, 2, 2) with nb = N//(2s)
        t = tw[:, :, l].reshape(ic, oc, N // (2 * s), s, 2, 2)
        # local slice for block g: global k = g*nb_loc + k_loc
        t = t.reshape(ic, oc, NBLK, nb_loc, s, 2, 2)
        # A view: (ic, oc, g, k_loc, 2, s, SB) rows evolve
        Av = A.reshape(ic, oc, NBLK, nb_loc, 2, s, SB)
        a0 = Av[:, :, :, :, 0]  # (ic,oc,g,k,s,SB)
        a1 = Av[:, :, :, :, 1]
        t00 = t[..., 0, 0, None]  # (ic,oc,g,k,s,1)
        t01 = t[..., 0, 1, None]
        t10 = t[..., 1, 0, None]
        t11 = t[..., 1, 1, None]
        new0 = t00 * a0 + t01 * a1
        new1 = t10 * a0 + t11 * a1
        Av[:, :, :, :, 0] = new0
        Av[:, :, :, :, 1] = new1
    return A


def _stageB_coeffs(tw):
    """Per-partition coefficient vectors for layers 8 and 9.

    Returns tb[ic, oc, 2(layer), 8(out chunk c'), 2(q), 128(p)] float32,
    with the 1/IC mean folded into layer 9, and the input-chunk index map.
    layer 8 (s=256): flat = k*512 + q*256 + j, j=(n7,p);  out chunk
      c' = (k, p_out, n7) -> reads chunks (k,q,n7), coeff t8[k, (n7,p), p_out, q].
    layer 9 (s=512): flat = q*512 + j, j=(n8,n7,p); out c' = (p_out, n8, n7)
      -> reads chunks (q, n8, n7), coeff t9[0, (n8,n7,p), p_out, q]/16.
    """
    ic, oc = tw.shape[0], tw.shape[1]
    t8 = tw[:, :, 8].reshape(ic, oc, 2, 256, 2, 2)   # [k, j, p_out, q]
    t9 = tw[:, :, 9].reshape(ic, oc, 1, 512, 2, 2)
    tb = np.zeros((ic, oc, 2, NCH, 2, P), dtype=np.float32)
    src = np.zeros((2, NCH, 2), dtype=np.int64)
    pr = np.arange(P)
    for cp in range(NCH):
        k, p_out, n7 = cp >> 2, (cp >> 1) & 1, cp & 1
        for q in range(2):
            tb[:, :, 0, cp, q] = t8[:, :, k, n7 * 128 + pr, p_out, q]
            src[0, cp, q] = k * 4 + q * 2 + n7
        p_out9, n8, n7_ = cp >> 2, (cp >> 1) & 1, cp & 1
        for q in range(2):
            tb[:, :, 1, cp, q] = t9[:, :, 0, (cp & 3) * 128 + pr, p_out9, q] / IC
            src[1, cp, q] = q * 4 + (cp & 3)
    return tb, src


_SRC8 = None  # filled lazily (static chunk-index map, twiddle-independent)


def _prep_host(x, twiddle, bias):
    """All host-side layout work. Returns per-core input maps (numpy)."""
    perm = _bitrev(N)
    y = np.ascontiguousarray(x).reshape(IC, B, N)[:, :, perm]
    # device layout y[ic, p, c*64+b]
    y_dev = np.ascontiguousarray(
        y.reshape(IC, B, NCH, P).transpose(0, 3, 2, 1)
    ).reshape(IC, P, NCH * B)

    A = _compose_stageA(np.asarray(twiddle, dtype=np.float32))
    tb, src = _stageB_coeffs(np.asarray(twiddle, dtype=np.float32))

    bias_np = np.asarray(bias, dtype=np.float32).reshape(OC, NCH, P)

    in_maps = []
    for core in range(NCORES):
        osl = slice(core * OCL, (core + 1) * OCL)
        Ac = A[:, osl]  # (IC, OCL, 4, 256, 256)
        # lhsT tiles: w[ic,o,p_k, g, h, kin, m] = Ac[ic,o,g][h*128+m, kin*128+p_k]
        w = np.ascontiguousarray(
            Ac.reshape(IC, OCL, NBLK, 2, P, 2, P)  # [g, h, m, kin, k]
            .transpose(0, 1, 6, 2, 3, 5, 4)        # [ic,o,k,g,h,kin,m]
        ).astype(W_DTYPE)
        tbc = np.ascontiguousarray(
            tb[:, osl].transpose(0, 1, 5, 2, 3, 4)  # [ic,o,p,l,c',q]
        ).reshape(IC, OCL, P, 32).astype(np.float32)
        bc = np.ascontiguousarray(
            np.broadcast_to(
                bias_np[osl].transpose(0, 2, 1)[:, :, :, None], (OCL, P, NCH, B)
            )
        ).reshape(OCL, P, NCH * B).astype(np.float32)
        in_maps.append(
            {
                "y": y_dev.astype(W_DTYPE),
                "w": w.reshape(IC, OCL, P, NBLK * 2 * 2 * P),
                "tb": tbc,
                "bias": bc,
            }
        )
    return in_maps, src


def _emulate_core(im, src):
    """Numpy emulation of the device program (for validating layout math)."""
    y = im["y"].astype(np.float32)      # (IC, 128, 512)
    w = im["w"].astype(np.float32).reshape(IC, OCL, P, NBLK, 2, 2, P)
    tb = im["tb"].reshape(IC, OCL, P, 2, NCH, 2)
    out = np.array(im["bias"], dtype=np.float32).reshape(OCL, P, NCH, B).copy()
    for o in range(OCL):
        for ic in range(IC):
            z = np.zeros((P, NCH, B), dtype=np.float32)
            yv = y[ic].reshape(P, NCH, B)
            for g in range(NBLK):
                for h in range(2):
                    acc = np.zeros((P, B), dtype=np.float32)
                    for kin in range(2):
                        lhsT = w[ic, o, :, g, h, kin]  # [k, m]
                        acc += lhsT.T @ yv[:, 2 * g + kin]
                    z[:, 2 * g + h] = acc
            y8 = np.zeros_like(z)
            for cp in range(NCH):
                y8[:, cp] = (
                    tb[ic, o, :, 0, cp, 0, None] * z[:, src[0, cp, 0]]
                    + tb[ic, o, :, 0, cp, 1, None] * z[:, src[0, cp, 1]]
                )
            for cp in range(NCH):
                out[o, :, cp] += (
                    tb[ic, o, :, 1, cp, 0, None] * y8[:, src[1, cp, 0]]
                    + tb[ic, o, :, 1, cp, 1, None] * y8[:, src[1, cp, 1]]
                )
    return out.reshape(OCL, P, NCH * B)


def _build_program(src):
    import concourse.bacc as bacc
    import concourse.mybir as mybir
    from concourse.tile import TileContext

    wdt = mybir.dt.bfloat16 if W_DTYPE != np.float32 else mybir.dt.float32
    f32 = mybir.dt.float32
    MULT, ADD = mybir.AluOpType.mult, mybir.AluOpType.add

    nc = bacc.Bacc(None, target_bir_lowering=False)
    y_d = nc.dram_tensor("y", (IC, P, NCH * B), wdt, kind="ExternalInput")
    w_d = nc.dram_tensor("w", (IC, OCL, P, NBLK * 4 * P), wdt, kind="ExternalInput")
    tb_d = nc.dram_tensor("tb", (IC, OCL, P, 32), f32, kind="ExternalInput")
    bias_d = nc.dram_tensor("bias", (OCL, P, NCH * B), f32, kind="ExternalInput")
    o_d = nc.dram_tensor("o", (OCL, P, NCH * B), f32, kind="ExternalOutput")

    with TileContext(nc) as tc:
        with (
            tc.tile_pool(name="ypool", bufs=2) as ypool,
            tc.tile_pool(name="wpool", bufs=3) as wpool,
            tc.tile_pool(name="tbpool", bufs=3) as tbpool,
            tc.tile_pool(name="accpool", bufs=OCL) as accpool,
            tc.tile_pool(name="y8pool", bufs=3) as y8pool,
            tc.tile_pool(name="psum", bufs=4, space="PSUM") as pspool,
        ):
            accs = []
            for o in range(OCL):
                acc = accpool.tile([P, NCH * B], f32, tag="acc")
                nc.sync.dma_start(out=acc[:], in_=bias_d[o])
                accs.append(acc)
            for ic in range(IC):
                ytile = ypool.tile([P, NCH * B], wdt)
                nc.sync.dma_start(out=ytile[:], in_=y_d[ic])
                for o in range(OCL):
                    wtile = wpool.tile([P, NBLK * 4 * P], wdt)
                    nc.sync.dma_start(out=wtile[:], in_=w_d[ic, o])
                    tbt = tbpool.tile([P, 32], f32)
                    nc.sync.dma_start(out=tbt[:], in_=tb_d[ic, o])
                    z = pspool.tile([P, NCH * B], f32)
                    for g in range(NBLK):
                        for h in range(2):
                            cp = 2 * g + h
                            for kin in range(2):
                                wi = ((g * 2 + h) * 2 + kin) * P
                                nc.tensor.matmul(
                                    z[:, cp * B : (cp + 1) * B],
                                    wtile[:, wi : wi + P],
                                    ytile[:, (2 * g + kin) * B : (2 * g + kin + 1) * B],
                                    start=(kin == 0),
                                    stop=(kin == 1),
                                )
                    y8 = y8pool.tile([P, NCH * B], f32)
                    for cp in range(NCH):
                        s0, s1 = int(src[0, cp, 0]), int(src[0, cp, 1])
                        osl = slice(cp * B, (cp + 1) * B)
                        # y8_cp = t_q1 * z_s1 ; then y8_cp = (z_s0*t_q0) + y8_cp
                        nc.vector.tensor_scalar_mul(
                            y8[:, osl],
                            z[:, s1 * B : (s1 + 1) * B],
                            tbt[:, (0 * NCH + cp) * 2 + 1 : (0 * NCH + cp) * 2 + 2],
                        )
                        nc.vector.scalar_tensor_tensor(
                            y8[:, osl],
                            z[:, s0 * B : (s0 + 1) * B],
                            tbt[:, (0 * NCH + cp) * 2 : (0 * NCH + cp) * 2 + 1],
                            y8[:, osl],
                            MULT,
                            ADD,
                        )
                    yo = y8pool.tile([P, NCH * B], f32, tag="yo")
                    for cp in range(NCH):
                        s0, s1 = int(src[1, cp, 0]), int(src[1, cp, 1])
                        osl = slice(cp * B, (cp + 1) * B)
                        nc.vector.tensor_scalar_mul(
                            yo[:, osl],
                            y8[:, s1 * B : (s1 + 1) * B],
                            tbt[:, (1 * NCH + cp) * 2 + 1 : (1 * NCH + cp) * 2 + 2],
                        )
                        nc.vector.scalar_tensor_tensor(
                            yo[:, osl],
                            y8[:, s0 * B : (s0 + 1) * B],
                            tbt[:, (1 * NCH + cp) * 2 : (1 * NCH + cp) * 2 + 1],
                            yo[:, osl],
                            MULT,
                            ADD,
                        )
                    nc.vector.tensor_add(accs[o][:], accs[o][:], yo[:])
            for o in range(OCL):
                nc.sync.dma_start(out=o_d[o], in_=accs[o][:])
    nc.finalize()
    return nc


_LAST_RESULTS = {"exec_time_ns": None}


def kernel(x, twiddle, bias, _trace=False, _emulate=False):
    in_maps, src = _prep_host(
        np.asarray(x), np.asarray(twiddle), np.asarray(bias)
    )
    if _emulate:
        outs = [_emulate_core(im, src) for im in in_maps]
    else:
        from concourse.bass_utils import run_bass_kernel_spmd

        nc = _build_program(src)
        res = run_bass_kernel_spmd(
            nc, in_maps, list(range(NCORES)), trace=_trace
        )
        _LAST_RESULTS["exec_time_ns"] = res.exec_time_ns
        _LAST_RESULTS["mean_exec_time_ns"] = res.mean_exec_time_ns
        outs = [r["o"] for r in res.results]
    # o[oc_l, p, c*64+b] -> (OC, B, N); final (B,OC,H,W) is a pure
    # reinterpret of (OC,B,N) bytes (reference uses .reshape, not transpose).
    full = np.concatenate(
        [
            np.asarray(o, dtype=np.float32)
            .reshape(OCL, P, NCH, B)
            .transpose(0, 3, 2, 1)
            .reshape(OCL, B, N)
            for o in outs
        ],
        axis=0,
    )
    return np.ascontiguousarray(full).reshape(B, OC, H, W).astype(np.float32)
